# revision 37
# baseline (speedup 1.0000x reference)
"""Trainium2 Bass kernel: DeepseekV4 CSA Compressor.

Math (per batch b):
  kv = hidden @ w_kv, gate = hidden @ w_gate          [S, 256]
  windows w = 0..S/32-1: tokens [w*32-32, w*32+32)  (prev block -> lo
  channels, current block -> hi channels; window 0 prev = 0 kv / -1e9 gate)
  pooled[w] = sum_j softmax_j(win_g + pos_bias)[j, d] * win_kv[j, d]
  RoPE on trailing 64 dims at position w*32.

Sharding: 8 cores = (4 batches) x (2 sequence halves).  Each core gets its
4096-token chunk transposed on host ([H, 4128] with a 32-token halo column
block in front; zeros for the first half, so the -1e9 gate fill is applied
via a per-core bias variant on the first window group).  No collectives.

Matmuls run in float16 (same 1.0 cycles/row PE rate as f32r but half the
HBM traffic).  Structure: per pair of 512-token groups, one fp16 DMA per
2 k-tiles covers both groups' column spans; matmuls run group-sequentially
(all 128 matmuls of group a, then group b from the same SBUF tiles), so
each group's 4 PSUM banks finish mid-pair and its softmax pooling overlaps
the next group's matmuls.  PSUM is read directly by the pooling ops (no
copy-out).  RoPE runs after a single end-of-kernel PE transpose to
[nwin, d] (pair mixing needs free-dim stride-2 APs; partition-base-crossing
DVE operands are rejected by walrus codegen).
"""

import numpy as np

HEAD_DIM = 128
ROPE_DIM = 64
RATIO = 32
ROPE_THETA = 10000.0
NEG = -1e9

B, S, H = 4, 8192, 4096
N_CORES = 8
HALF = S // 2                 # tokens per core
NWIN_CORE = HALF // RATIO     # windows per core = 128
GW = 512                      # tokens per matmul/pooling group
WPG = GW // RATIO             # windows per group = 16

_CACHE: dict = {}


def build_program(T_main: int, H_: int, nwin: int):
    """Build the single-core SPMD Bass program.

    T_main: tokens per core (multiple of 2*GW); H_: hidden dim (multiple of
    512); nwin: windows per core (= T_main // RATIO).
    """
    from contextlib import ExitStack

    import concourse.bacc as bacc
    import concourse.mybir as mybir
    import concourse.tile as tile

    f32 = mybir.dt.float32
    f16 = mybir.dt.float16
    AF = mybir.ActivationFunctionType
    AX = mybir.AxisListType

    d = HEAD_DIM
    r = RATIO
    hw = ROPE_DIM // 2        # 32 rotary pairs
    nope = d - ROPE_DIM       # 64
    NG = T_main // GW         # groups
    KT = H_ // 128            # k tiles
    C = 4 * d                 # 512 projection channels (kv_lo|kv_hi|g_lo|g_hi)
    NPAIR = NG // 2
    KKT = KT // 2
    NQ = KT // 4              # weight quads
    PW = 2 * GW + r           # 1056 columns per pair load
    assert NG % 2 == 0 and KT % 4 == 0

    nc = bacc.Bacc("TRN2", target_bir_lowering=False, debug=False,
                   num_devices=N_CORES)
    # Pre-tiled on host in exact consumption order: each [128, 2, PW] block
    # is one fully-contiguous ~0.5MB DMA (sequential HBM streaming).
    hTp = nc.dram_tensor("hTp", [NPAIR, KKT, 128, 2, PW], f16,
                         kind="ExternalInput").ap()
    W4 = nc.dram_tensor("W4", [NQ, 128, 4, C], f16, kind="ExternalInput").ap()
    bias_lo = nc.dram_tensor("bias_lo", [d, GW], f32, kind="ExternalInput").ap()
    bias_lo0 = nc.dram_tensor("bias_lo0", [d, GW], f32, kind="ExternalInput").ap()
    bias_hi = nc.dram_tensor("bias_hi", [d, GW], f32, kind="ExternalInput").ap()
    SPLIT = nwin - WPG            # windows finalized before the last group
    cos_in = nc.dram_tensor("cosp", [SPLIT, hw], f32, kind="ExternalInput").ap()
    sin_in = nc.dram_tensor("sinp", [SPLIT, hw], f32, kind="ExternalInput").ap()
    cosB_in = nc.dram_tensor("cospB", [WPG, hw], f32, kind="ExternalInput").ap()
    sinB_in = nc.dram_tensor("sinpB", [WPG, hw], f32, kind="ExternalInput").ap()
    ident = nc.dram_tensor("ident", [d, d], f32, kind="ExternalInput").ap()
    out = nc.dram_tensor("out", [nwin, d], f32, kind="ExternalOutput").ap()

    with tile.TileContext(nc) as tc, ExitStack() as ctx:
        wp = ctx.enter_context(tc.tile_pool(name="wp", bufs=1))
        hp = ctx.enter_context(tc.tile_pool(name="hp", bufs=1))
        pp = ctx.enter_context(tc.tile_pool(name="pp", bufs=2, space="PSUM"))
        sp = ctx.enter_context(tc.tile_pool(name="sp", bufs=2))
        smp = ctx.enter_context(tc.tile_pool(name="smp", bufs=2))
        cp = ctx.enter_context(tc.tile_pool(name="cp", bufs=1))

        # Stationary weights, 4 k-tiles per DMA, all on gpsimd (SWDGE) to
        # keep the two HWDGE queues (sync, scalar) free for the stream.
        w_sb = []
        for q in range(NQ):
            w_q = wp.tile([128, 4, C], f16, tag=f"w{q}", name=f"w{q}")
            nc.gpsimd.dma_start(w_q[:], W4[q])
            w_sb.append(w_q)

        def wv(k):
            return w_sb[k // 4][:, k % 4, :]

        blo = cp.tile([d, GW], f32, tag="blo")
        nc.gpsimd.dma_start(blo[:], bias_lo[:])
        blo0 = cp.tile([d, GW], f32, tag="blo0")
        nc.gpsimd.dma_start(blo0[:], bias_lo0[:])
        bhi = cp.tile([d, GW], f32, tag="bhi")
        nc.gpsimd.dma_start(bhi[:], bias_hi[:])
        cosb = cp.tile([SPLIT, hw], f32, tag="cosb")
        nc.gpsimd.dma_start(cosb[:], cos_in[:])
        sinb = cp.tile([SPLIT, hw], f32, tag="sinb")
        nc.gpsimd.dma_start(sinb[:], sin_in[:])
        cosbB = cp.tile([WPG, hw], f32, tag="cosbB")
        nc.gpsimd.dma_start(cosbB[:], cosB_in[:])
        sinbB = cp.tile([WPG, hw], f32, tag="sinbB")
        nc.gpsimd.dma_start(sinbB[:], sinB_in[:])
        idt = cp.tile([d, d], f32, tag="idt")
        nc.gpsimd.dma_start(idt[:], ident[:])

        pooled = cp.tile([d, nwin], f32, tag="pooled")

        def pooling_group(g, ps, copy_kv=False):
            # Softmax-gated pooling for one group, reading kv/gate PSUM
            # banks directly.  No max-subtraction: gate values are O(5), exp
            # is safe, and the -1e9 first-window fill underflows to 0.
            # copy_kv: copy the kv banks to SBUF first so they free quickly
            # (used for pair 0, whose banks are needed right at its end).
            gsl = slice(g * WPG, (g + 1) * WPG)
            kvlo_s, kvhi_s = ps[0], ps[1]
            if copy_kv:
                kvlo_s = sp.tile([d, GW], f32, tag="kvloS", name=f"kvloS{g}")
                nc.vector.tensor_copy(kvlo_s[:], ps[0][:])
                kvhi_s = sp.tile([d, GW], f32, tag="kvhiS", name=f"kvhiS{g}")
                nc.vector.tensor_copy(kvhi_s[:], ps[1][:])
            tglo = sp.tile([d, GW], f32, tag="tglo", name=f"tglo{g}")
            nc.vector.tensor_add(tglo[:], ps[2][:],
                                 (blo0 if g == 0 else blo)[:])
            tghi = sp.tile([d, GW], f32, tag="tghi", name=f"tghi{g}")
            nc.vector.tensor_add(tghi[:], ps[3][:], bhi[:])
            elo = sp.tile([d, GW], f32, tag="elo", name=f"elo{g}")
            nc.scalar.activation(elo[:], tglo[:], AF.Exp)
            ehi = sp.tile([d, GW], f32, tag="ehi", name=f"ehi{g}")
            nc.scalar.activation(ehi[:], tghi[:], AF.Exp)
            slo = smp.tile([d, WPG], f32, tag="slo", name=f"slo{g}")
            nc.vector.reduce_sum(
                slo[:], elo[:].rearrange("p (w j) -> p w j", j=r), axis=AX.X)
            shi = smp.tile([d, WPG], f32, tag="shi", name=f"shi{g}")
            nc.vector.reduce_sum(
                shi[:], ehi[:].rearrange("p (w j) -> p w j", j=r), axis=AX.X)
            sall = smp.tile([d, WPG], f32, tag="sall", name=f"sall{g}")
            nc.gpsimd.tensor_add(sall[:], slo[:], shi[:])
            plo = sp.tile([d, GW], f32, tag="plo", name=f"plo{g}")
            nc.vector.tensor_mul(plo[:], elo[:], kvlo_s[:])
            phi = sp.tile([d, GW], f32, tag="phi", name=f"phi{g}")
            nc.vector.tensor_mul(phi[:], ehi[:], kvhi_s[:])
            nlo = smp.tile([d, WPG], f32, tag="nlo", name=f"nlo{g}")
            nc.vector.reduce_sum(
                nlo[:], plo[:].rearrange("p (w j) -> p w j", j=r), axis=AX.X)
            nhi = smp.tile([d, WPG], f32, tag="nhi", name=f"nhi{g}")
            nc.vector.reduce_sum(
                nhi[:], phi[:].rearrange("p (w j) -> p w j", j=r), axis=AX.X)
            num = smp.tile([d, WPG], f32, tag="num", name=f"num{g}")
            nc.vector.tensor_add(num[:], nlo[:], nhi[:])
            rs = smp.tile([d, WPG], f32, tag="rs", name=f"rs{g}")
            nc.vector.reciprocal(rs[:], sall[:])
            nc.vector.tensor_mul(pooled[:, gsl], num[:], rs[:])

        def rope_block(ptr_t, cos_t, sin_t, rows, row0):
            # RoPE on [rows, d] with free-dim stride-2 pair mixing, then the
            # output DMA for those window rows.
            outsb = cp.tile([rows, d], f32, tag=f"outsb{row0}")
            nc.vector.tensor_copy(outsb[:, 0:nope], ptr_t[:, 0:nope])
            rp = ptr_t[:, nope:d].rearrange("p (a two) -> p a two", two=2)
            re_, ro_ = rp[:, :, 0], rp[:, :, 1]
            op = outsb[:, nope:d].rearrange("p (a two) -> p a two", two=2)
            oe_, oo_ = op[:, :, 0], op[:, :, 1]
            t1 = smp.tile([rows, hw], f32, tag=f"t1_{row0}")
            t2 = smp.tile([rows, hw], f32, tag=f"t2_{row0}")
            nc.vector.tensor_mul(t1[:], re_, cos_t[:])
            nc.vector.tensor_mul(t2[:], ro_, sin_t[:])
            nc.vector.tensor_sub(oe_, t1[:], t2[:])
            t3 = smp.tile([rows, hw], f32, tag=f"t3_{row0}")
            t4 = smp.tile([rows, hw], f32, tag=f"t4_{row0}")
            nc.vector.tensor_mul(t3[:], ro_, cos_t[:])
            nc.vector.tensor_mul(t4[:], re_, sin_t[:])
            nc.vector.tensor_add(oo_, t3[:], t4[:])
            nc.sync.dma_start(out[row0:row0 + rows, :], outsb[:])

        def maybe_tail(g):
            # Windows of groups 0..NG-2 finalize while the last group's
            # matmuls run; only the last group's 16 windows are in the tail.
            if g == NG - 2:
                ptrA = pp.tile([SPLIT, d], f32, tag="glo", name="ptrA")
                nc.tensor.transpose(ptrA[:], pooled[:, 0:SPLIT], idt[:])
                rope_block(ptrA, cosb, sinb, SPLIT, 0)
            elif g == NG - 1:
                ptrB = pp.tile([WPG, d], f32, tag="kvhi", name="ptrB")
                nc.tensor.transpose(ptrB[:], pooled[:, SPLIT:nwin], idt[:])
                rope_block(ptrB, cosbB, sinbB, WPG, SPLIT)

        # Per pair: one fp16 DMA per 2 k-tiles covers both groups' column
        # spans.  Pair 0 interleaves both groups per k-tile (half the
        # stream-tile burn rate, riding out the weight-preload bus burst);
        # later pairs run group-sequentially from the same 16 SBUF tiles so
        # each group's pooling overlaps the next group's matmuls.
        for p in range(NPAIR):
            def issue_ht(kk):
                ht2 = hp.tile([128, 2, PW], f16, tag=f"ht{kk}",
                              name=f"ht{kk}_{p}")
                dma_eng = nc.sync if kk % 2 == 0 else nc.scalar
                dma_eng.dma_start(ht2[:], hTp[p, kk])
                return ht2

            if p == 0:
                hts = [issue_ht(kk) for kk in range(KKT)]
                pss = [[pp.tile([d, GW], f32, tag=t, name=f"{t}_{g}")
                        for t in ("kvlo", "kvhi", "glo", "ghi")]
                       for g in (0, 1)]
                for kk in range(KKT):
                    for j in range(2):
                        k = 2 * kk + j
                        ht_k = hts[kk][:, j, :]
                        st, sp_ = (k == 0), (k == KT - 1)
                        w_k = wv(k)
                        for half in range(2):
                            off = half * GW
                            ps = pss[half]
                            nc.tensor.matmul(ps[0][:], w_k[:, 0:d],
                                             ht_k[:, off:off + GW],
                                             start=st, stop=sp_)
                            nc.tensor.matmul(ps[1][:], w_k[:, d:2 * d],
                                             ht_k[:, off + r:off + GW + r],
                                             start=st, stop=sp_)
                            nc.tensor.matmul(ps[2][:], w_k[:, 2 * d:3 * d],
                                             ht_k[:, off:off + GW],
                                             start=st, stop=sp_)
                            nc.tensor.matmul(ps[3][:], w_k[:, 3 * d:4 * d],
                                             ht_k[:, off + r:off + GW + r],
                                             start=st, stop=sp_)
                for half in range(2):
                    pooling_group(half, pss[half], copy_kv=True)
                    maybe_tail(half)
                continue
            hts = [issue_ht(kk) for kk in range(KKT)]
            for half in range(2):
                g = 2 * p + half
                off = half * GW
                ps = [pp.tile([d, GW], f32, tag=t, name=f"{t}_{g}")
                      for t in ("kvlo", "kvhi", "glo", "ghi")]
                for kk in range(KKT):
                    for j in range(2):
                        k = 2 * kk + j
                        ht_k = hts[kk][:, j, :]
                        st, sp_ = (k == 0), (k == KT - 1)
                        w_k = wv(k)
                        nc.tensor.matmul(ps[0][:], w_k[:, 0:d],
                                         ht_k[:, off:off + GW],
                                         start=st, stop=sp_)
                        nc.tensor.matmul(ps[1][:], w_k[:, d:2 * d],
                                         ht_k[:, off + r:off + GW + r],
                                         start=st, stop=sp_)
                        nc.tensor.matmul(ps[2][:], w_k[:, 2 * d:3 * d],
                                         ht_k[:, off:off + GW],
                                         start=st, stop=sp_)
                        nc.tensor.matmul(ps[3][:], w_k[:, 3 * d:4 * d],
                                         ht_k[:, off + r:off + GW + r],
                                         start=st, stop=sp_)
                pooling_group(g, ps)
                maybe_tail(g)

    nc.compile()
    return nc


def _host_inputs(hidden_states, w_kv, w_gate, position_bias,
                 T_main: int, nwin: int, n_cores: int):
    """Build per-core input maps (list of dicts) for the SPMD program."""
    d, r = HEAD_DIM, RATIO
    hww = ROPE_DIM // 2
    H_ = hidden_states.shape[2]
    KT = H_ // 128
    NQ = KT // 4
    C = 4 * d

    w_kv = np.asarray(w_kv, np.float32)
    w_gate = np.asarray(w_gate, np.float32)
    Wfull = np.concatenate([w_kv, w_gate], axis=1)
    Wr = Wfull.astype(np.float16)
    W4 = np.ascontiguousarray(
        Wr.reshape(NQ, 4, 128, C).transpose(0, 2, 1, 3))

    biasT = np.ascontiguousarray(
        np.asarray(position_bias, np.float32).T)         # [d, 2r]
    bias_lo_t = np.ascontiguousarray(np.tile(biasT[:, :r], (1, WPG)))
    bias_hi_t = np.ascontiguousarray(np.tile(biasT[:, r:], (1, WPG)))
    bias_lo_g0 = bias_lo_t.copy()
    bias_lo_g0[:, :r] = NEG

    inv_freq = 1.0 / (ROPE_THETA ** (
        np.arange(0, ROPE_DIM, 2, dtype=np.float32) / ROPE_DIM))  # [32]
    ident = np.eye(d, dtype=np.float32)

    hs = np.asarray(hidden_states, np.float32)
    halves_per_batch = n_cores // hs.shape[0]
    NPAIR = T_main // (2 * GW)
    KKT = KT // 2
    PW = 2 * GW + r
    in_maps = []
    for c in range(n_cores):
        b, hf = c // halves_per_batch, c % halves_per_batch
        start = hf * T_main
        chunk = np.empty((H_, T_main + r), np.float16)
        chunk[:, r:] = hs[b, start:start + T_main].T
        if hf == 0:
            chunk[:, :r] = 0.0
        else:
            chunk[:, :r] = hs[b, start - r:start].T
        # Pre-tile into exact DMA consumption order:
        # hTp[pair, kk, p, j, c] = chunk[(2kk+j)*128 + p, pair*1024 + c]
        v = chunk.reshape(KKT, 2, 128, T_main + r)
        hTp = np.ascontiguousarray(
            np.stack([v[:, :, :, p0 * 2 * GW:p0 * 2 * GW + PW]
                      for p0 in range(NPAIR)], axis=0).transpose(0, 1, 3, 2, 4))
        w0 = hf * nwin
        positions = (w0 + np.arange(nwin, dtype=np.float32)) * r
        freqs = positions[:, None] * inv_freq[None, :]     # [nwin, 32]
        cosf = np.cos(freqs).astype(np.float32)
        sinf = np.sin(freqs).astype(np.float32)
        split = nwin - WPG
        in_maps.append({
            "hTp": hTp,
            "W4": W4,
            "bias_lo": bias_lo_t,
            "bias_lo0": bias_lo_g0 if hf == 0 else bias_lo_t,
            "bias_hi": bias_hi_t,
            "cosp": np.ascontiguousarray(cosf[:split]),
            "sinp": np.ascontiguousarray(sinf[:split]),
            "cospB": np.ascontiguousarray(cosf[split:]),
            "sinpB": np.ascontiguousarray(sinf[split:]),
            "ident": ident,
        })
    return in_maps


def kernel(hidden_states, w_kv, w_gate, position_bias, _want_profile=False):
    """Full-input, full-output entry point.  Shards over 8 NeuronCores."""
    from concourse.bass_utils import run_bass_kernel_spmd

    hs = np.asarray(hidden_states, np.float32)
    B_, S_, H_ = hs.shape
    n = S_ // RATIO
    if "nc" not in _CACHE:
        _CACHE["nc"] = build_program(HALF, H_, NWIN_CORE)
    nc = _CACHE["nc"]

    in_maps = _host_inputs(hs, w_kv, w_gate, position_bias,
                           HALF, NWIN_CORE, N_CORES)
    kwargs = {}
    if _want_profile:
        import os
        import shutil

        shutil.rmtree("work/prof", ignore_errors=True)
        os.makedirs("work/prof", exist_ok=True)
        kwargs = {"trace": True, "tmpdir": os.path.abspath("work/prof")}
    res = run_bass_kernel_spmd(nc, in_maps, list(range(N_CORES)), **kwargs)

    out = np.empty((B_, n, HEAD_DIM), np.float32)
    halves_per_batch = N_CORES // B_
    for c in range(N_CORES):
        b, hf = c // halves_per_batch, c % halves_per_batch
        out[b, hf * NWIN_CORE:(hf + 1) * NWIN_CORE] = res.results[c]["out"]
    if _want_profile:
        return out, res
    return out
